# revision 12
# baseline (speedup 1.0000x reference)
"""Trainium2 Bass kernel v3 for nn_CustomMultiheadAttention (linear attention
with low-rank QKV projections), SPMD over 8 NeuronCores.

Sharding: (batch, seq-half) -> core; pairwise AllReduce of kv between the two
cores sharing a batch.

v3 changes vs v2:
- fp8e4 DoubleRow matmuls for the four big GEMMs (K/V/Q projections and the
  out-projection): 2 contraction rows per PE cycle.  Inputs scaled by 4,
  weights by 32 (product scale S=128) to sit in e4m3's normal range.
- Exact feature-map identity elu(t)+1 = max(exp(min(t,0)), 1+t), computed in
  3 elementwise ops (TS min / Exp / STT max).  Feature maps are kept SCALED
  by S: the scale cancels exactly in num/denom (and v's scale is removed in
  the tiny kvbd fixup), so no descale passes over the big tensors.
- Projection biases folded into PSUM via 1-partition bias matmuls (K, out)
  or per-partition activation bias pointers (Q, via exp(x + ln S) = S e^x).
  V's bias is reconstructed after the AllReduce as ksum (x) vu_b, a rank-1
  fixup on the tiny kv tile.
- AllReduce payload in bf16 (halved); collective issued on Pool whose
  post-collective program-order work (kvbd fixup) needs the result anyway.
- Out-projection: bias matmul + single TS descale from PSUM, y DMA'd per
  128-token block.
"""

import contextlib

import numpy as np

import concourse.bass as bass
import concourse.tile as tile
from concourse import bacc, mybir
from concourse.bass_utils import run_bass_kernel_spmd

F32 = mybir.dt.float32
F32R = mybir.dt.float32r
BF16 = mybir.dt.bfloat16
FP8 = mybir.dt.float8e4

B, S, E, H, R = 4, 4096, 1024, 16, 512
D = E // H  # 64
N_CORES = 8

SX = 4.0      # input scale for fp8
SW = 32.0     # weight scale for fp8
SC = SX * SW  # product scale S = 128
LNS = float(np.log(SC))
S_ATT = 1.0    # att tile scale (folded into ind); bf16 att
S_OW = 1.0     # out_w kept bf16
S_O = S_ATT * S_OW  # out psum scale


def build_nc(T, n_cores, groups):
    CH = min(512, T)        # tokens per chunk
    NCH = T // CH           # chunks
    TB = CH // 128          # 128-token blocks per chunk
    EC = E // 128           # 8
    FC5 = E // 512          # 2
    HP = H // 2             # 8 head pairs
    hpf = 512 // D          # heads per 512-feature group = 8

    nc = bacc.Bacc("TRN2", target_bir_lowering=False, debug=False,
                   num_devices=n_cores)

    xqT = nc.declare_dram_parameter("xqT", [E, T], FP8, isOutput=False).ap()
    xkT = nc.declare_dram_parameter("xkT", [E, T], FP8, isOutput=False).ap()
    xvT = nc.declare_dram_parameter("xvT", [E, T], FP8, isOutput=False).ap()
    # combined (up@down) projection weights, tile layout [128, EC(in), E(out)]
    wqc = nc.declare_dram_parameter("wqc", [128, EC, E], FP8, isOutput=False).ap()
    wkc = nc.declare_dram_parameter("wkc", [128, EC, E], FP8, isOutput=False).ap()
    wvc = nc.declare_dram_parameter("wvc", [128, EC, E], FP8, isOutput=False).ap()
    owT = nc.declare_dram_parameter("owT", [128, EC, E], BF16, isOutput=False).ap()
    # biases
    kb_row_d = nc.declare_dram_parameter("kb_row", [1, E], F32R, isOutput=False).ap()
    ob_row_d = nc.declare_dram_parameter("ob_row", [1, E], F32R, isOutput=False).ap()
    qb0_d = nc.declare_dram_parameter("qb0", [128, EC], F32, isOutput=False).ap()
    qb1_d = nc.declare_dram_parameter("qb1", [128, EC], F32, isOutput=False).ap()
    vbb_d = nc.declare_dram_parameter("vbb", [128, EC, D], F32, isOutput=False).ap()
    lns_d = nc.declare_dram_parameter("lns", [128, 1], F32, isOutput=False).ap()
    ones_d = nc.declare_dram_parameter("ones_row", [1, 128], F32R,
                                       isOutput=False).ap()
    ind_d = nc.declare_dram_parameter("ind", [16, EC, 128], F32R,
                                      isOutput=False).ap()
    y = nc.declare_dram_parameter("y", [T, E], F32, isOutput=True).ap()

    cc_in = nc.dram_tensor("cc_in", [128, HP, D + 1], BF16).ap()
    cc_out = nc.dram_tensor("cc_out", [128, HP, D + 1], BF16).ap()

    Exp = mybir.ActivationFunctionType.Exp
    Copy = mybir.ActivationFunctionType.Copy
    DR = mybir.MatmulPerfMode.DoubleRow
    add, mult = mybir.AluOpType.add, mybir.AluOpType.mult
    amin, amax = mybir.AluOpType.min, mybir.AluOpType.max
    sub = mybir.AluOpType.subtract

    with tile.TileContext(nc) as tc:
        with (
            tc.tile_pool(name="const", bufs=1) as const,
            tc.tile_pool(name="persist", bufs=1) as persist,
        ):
            kb_row = const.tile([1, E], F32R)
            qb0 = const.tile([128, EC], F32)
            qb1 = const.tile([128, EC], F32)
            k_ones = const.tile([1, 128], F32R)
            lns_t = const.tile([128, 1], F32)
            nc.sync.dma_start(out=kb_row[:], in_=kb_row_d[:, :])
            nc.sync.dma_start(out=qb0[:], in_=qb0_d[:, :])
            nc.sync.dma_start(out=qb1[:], in_=qb1_d[:, :])
            nc.sync.dma_start(out=k_ones[:], in_=ones_d[:, :])
            nc.sync.dma_start(out=lns_t[:], in_=lns_d[:, :])

            qfm_all = persist.tile([128, EC, T], BF16)  # S*phi(q), feat-major
            kv_acc = persist.tile([128, HP, D + 1], F32)  # S^2 kv | S ksum

            # Q-phase pools opened early so wqT + xq chunk 0 can prefetch
            # during the KV phase
            _stack = contextlib.ExitStack()
            wqp = _stack.enter_context(tc.tile_pool(name="wq", bufs=1))
            wqT = wqp.tile([128, EC, E], FP8, tag="wq")
            xqp = _stack.enter_context(tc.tile_pool(name="xq", bufs=2))
            xq_re = xqT.rearrange("(ec p) t -> p ec t", p=128)
            xq_t0 = xqp.tile([128, EC, CH], FP8, tag="xq")

            # ---------------- Phase KV ----------------
            with (
                tc.tile_pool(name="wkv", bufs=1) as wkvp,
                tc.tile_pool(name="xkv", bufs=2) as xkvp,
                tc.tile_pool(name="uppsum", bufs=3, space="PSUM") as ups,
                tc.tile_pool(name="ktv", bufs=2) as ktvp,
                tc.tile_pool(name="elu", bufs=3) as elup,
                tc.tile_pool(name="kvacc", bufs=2, space="PSUM") as kvap,
            ):
                xk_re = xkT.rearrange("(ec p) t -> p ec t", p=128)
                xv_re = xvT.rearrange("(ec p) t -> p ec t", p=128)

                # first x tiles ahead of the weights in the DMA queue
                xk_t0 = xkvp.tile([128, EC, 128], FP8, tag="xk")
                nc.sync.dma_start(out=xk_t0[:], in_=xk_re[:, :, 0:128])
                xv_t0 = xkvp.tile([128, EC, 128], FP8, tag="xv")
                nc.sync.dma_start(out=xv_t0[:], in_=xv_re[:, :, 0:128])

                wkT = wkvp.tile([128, EC, E], FP8, tag="wk")
                wvT = wkvp.tile([128, EC, E], FP8, tag="wv")
                # per-ec pieces so the first accumulation group can start
                # as soon as its slice lands
                for ec in range(EC):
                    nc.sync.dma_start(out=wkT[:, ec, 0:512],
                                      in_=wkc[:, ec, 0:512])
                nc.sync.dma_start(out=wvT[:, :, 0:512], in_=wvc[:, :, 0:512])
                nc.sync.dma_start(out=wkT[:, :, 512:E], in_=wkc[:, :, 512:E])
                nc.sync.dma_start(out=wvT[:, :, 512:E], in_=wvc[:, :, 512:E])

                for c in range(NCH):
                    if c == min(1, NCH - 1):
                        # prefetch the q weight once startup traffic is done
                        nc.sync.dma_start(out=wqT[:], in_=wqc[:, :, :])
                    if c == min(2, NCH - 1):
                        nc.sync.dma_start(out=xq_t0[:],
                                          in_=xq_re[:, :, 0:CH])
                    ktm = ktvp.tile([128, TB, H, D + 1], BF16, tag="ktm")
                    vtm = ktvp.tile([128, TB, H, D + 1], BF16, tag="vtm")
                    nc.vector.memset(vtm[:, :, :, D:D + 1], 1.0)

                    for tb in range(TB):
                        t0 = c * CH + tb * 128
                        if c == 0 and tb == 0:
                            xk_t, xv_t = xk_t0, xv_t0
                        else:
                            xk_t = xkvp.tile([128, EC, 128], FP8, tag="xk")
                            nc.sync.dma_start(out=xk_t[:],
                                              in_=xk_re[:, :, t0:t0 + 128])
                            xv_t = xkvp.tile([128, EC, 128], FP8, tag="xv")
                            nc.sync.dma_start(out=xv_t[:],
                                              in_=xv_re[:, :, t0:t0 + 128])

                        for fc in range(FC5):
                            fsl = bass.ds(fc * 512, 512)
                            # K: token-major proj; psum = S*(k~ + b + 1)
                            ps = ups.tile([128, 512], F32, tag="ups")
                            nc.tensor.matmul(ps[:], k_ones[:],
                                             kb_row[:, fsl],
                                             start=True, stop=False)
                            for e2 in range(EC // 2):
                                nc.tensor.matmul(
                                    ps[:],
                                    xk_t[:, 2 * e2:2 * e2 + 2, :],
                                    wkT[:, 2 * e2:2 * e2 + 2, fsl],
                                    start=False, stop=(e2 == EC // 2 - 1),
                                    perf_mode=DR)
                            # phi = max(exp(min(t,0)), 1+t), stored as S*phi
                            m = elup.tile([128, 512], BF16, tag="elu_m")
                            nc.vector.tensor_scalar(m[:], ps[:], SC, 0.0,
                                                    op0=sub, op1=amin)
                            e = elup.tile([128, 512], BF16, tag="elu_e")
                            nc.scalar.activation(e[:], m[:], Exp,
                                                 bias=lns_t[:],
                                                 scale=1.0 / SC)
                            dsl = ktm[:, tb, fc * hpf:(fc + 1) * hpf, 0:D]
                            nc.vector.scalar_tensor_tensor(
                                dsl, ps[:], 1.0,
                                e[:].rearrange("p (h f) -> p h f", h=hpf),
                                op0=mult, op1=amax)

                            # V: token-major proj, psum = S * (x @ wv)
                            psv = ups.tile([128, 512], F32, tag="ups")
                            for e2 in range(EC // 2):
                                nc.tensor.matmul(
                                    psv[:],
                                    xv_t[:, 2 * e2:2 * e2 + 2, :],
                                    wvT[:, 2 * e2:2 * e2 + 2, fsl],
                                    start=(e2 == 0),
                                    stop=(e2 == EC // 2 - 1),
                                    perf_mode=DR)
                            vsl = vtm[:, tb, fc * hpf:(fc + 1) * hpf, 0:D]
                            nc.scalar.activation(
                                vsl,
                                psv[:].rearrange("p (h f) -> p h f", h=hpf),
                                Copy)

                    # kv partial for this chunk, accumulate into kv_acc
                    for h in range(H):
                        pkv = kvap.tile([64, D + 1], F32, tag="pkv")
                        for tb in range(TB):
                            nc.tensor.matmul(
                                pkv[:], ktm[:, tb, h, 0:D],
                                vtm[:, tb, h, 0:D + 1],
                                start=(tb == 0), stop=(tb == TB - 1))
                        b0 = 64 * (h % 2)
                        acc_sl = kv_acc[b0:b0 + 64, h // 2, :]
                        if c == 0:
                            nc.vector.tensor_copy(acc_sl, pkv[:])
                        else:
                            nc.vector.tensor_add(acc_sl, acc_sl, pkv[:])

            # ---------------- AllReduce (overlapped with Q pass) ----------
            ccp = _stack.enter_context(tc.tile_pool(name="ccp", bufs=1))
            cc_sb = ccp.tile([128, HP, D + 1], BF16, tag="ccsb")
            nc.gpsimd.tensor_copy(cc_sb[:], kv_acc[:])
            nc.sync.dma_start(out=cc_in[:], in_=cc_sb[:])
            nc.gpsimd.collective_compute(
                "AllReduce", mybir.AluOpType.add,
                ins=[cc_in[:]], outs=[cc_out[:]],
                replica_groups=groups)

            # ------- phase-2 constants: start right after the collective ---
            w2p = _stack.enter_context(tc.tile_pool(name="w2", bufs=1))
            kvxp = _stack.enter_context(tc.tile_pool(name="kvx", bufs=1))

            # indicator tiles (carry S_ATT) for the denom broadcast matmul
            ind = kvxp.tile([16, EC, 128], F32R, tag="ind")
            nc.sync.dma_start(out=ind[:], in_=ind_d[:, :, :])
            vbb = kvxp.tile([128, EC, D], F32, tag="vbb")
            nc.sync.dma_start(out=vbb[:], in_=vbb_d[:, :, :])

            kv_red = kvxp.tile([128, HP, D + 1], BF16, tag="kvred")
            nc.sync.dma_start(out=kv_red[:], in_=cc_out[:])

            owt = w2p.tile([128, EC, E], BF16, tag="ow")
            ob_row = w2p.tile([1, E], F32R, tag="ob")

            # block-diag kv tiles [128, 128] per head pair + ksum tiles
            # kvbd = S*kv_true = kv_raw/S + ksum_raw (x) vu_b
            kvbd = kvxp.tile([128, EC, 128], BF16, tag="kvbd")
            nc.vector.memset(kvbd[:], 0.0)
            ksb = kvxp.tile([128, EC, 16], BF16, tag="ksb")
            nc.vector.memset(ksb[:], 0.0)
            outr = kvxp.tile([128, EC, D], F32, tag="outr")
            ksc = kvxp.tile([128, HP], F32, tag="ksc")
            nc.gpsimd.tensor_copy(
                ksc[:], kv_red[:, :, D:D + 1].rearrange("p h one -> p (h one)"))
            for ec in range(EC):
                nc.vector.tensor_scalar(
                    outr[:, ec, :], vbb[:, ec, :],
                    ksc[:, ec:ec + 1], None, op0=mult)
                nc.vector.scalar_tensor_tensor(
                    kvbd[0:64, ec, 0:64], kv_red[0:64, ec, 0:D],
                    1.0 / SC, outr[0:64, ec, :], op0=mult, op1=add)
                nc.vector.scalar_tensor_tensor(
                    kvbd[64:128, ec, 64:128], kv_red[64:128, ec, 0:D],
                    1.0 / SC, outr[64:128, ec, :], op0=mult, op1=add)
                nc.gpsimd.tensor_copy(ksb[0:64, ec, 2 * ec:2 * ec + 1],
                                      kv_red[0:64, ec, D:D + 1])
                nc.gpsimd.tensor_copy(
                    ksb[64:128, ec, 2 * ec + 1:2 * ec + 2],
                    kv_red[64:128, ec, D:D + 1])

            # ---------------- Q feature-map pass --------------------------
            with (
                tc.tile_pool(name="qpsum", bufs=3, space="PSUM") as qps,
                tc.tile_pool(name="elu2", bufs=3) as elu2,
            ):
                for c in range(NCH):
                    if c == min(1, NCH - 1):
                        # prefetch phase-2 weights during the q pass
                        nc.sync.dma_start(out=owt[:], in_=owT[:, :, :])
                        nc.sync.dma_start(out=ob_row[:], in_=ob_row_d[:, :])
                    if c == 0:
                        xq_t = xq_t0
                    else:
                        xq_t = xqp.tile([128, EC, CH], FP8, tag="xq")
                        nc.sync.dma_start(
                            out=xq_t[:],
                            in_=xq_re[:, :, c * CH:(c + 1) * CH])
                    for fo in range(EC):
                        ps = qps.tile([128, CH], F32, tag="qps")
                        for e2 in range(EC // 2):
                            nc.tensor.matmul(
                                ps[:],
                                wqT[:, 2 * e2:2 * e2 + 2,
                                    fo * 128:(fo + 1) * 128],
                                xq_t[:, 2 * e2:2 * e2 + 2, :],
                                start=(e2 == 0), stop=(e2 == EC // 2 - 1),
                                perf_mode=DR)
                        # psum = S*q~ ; bias via per-partition pointers
                        m = elu2.tile([128, CH], F32, tag="m2")
                        nc.vector.tensor_scalar(
                            m[:], ps[:], qb0[:, fo:fo + 1], 0.0,
                            op0=add, op1=amin)
                        e = elu2.tile([128, CH], BF16, tag="e2")
                        nc.scalar.activation(e[:], m[:], Exp,
                                             bias=lns_t[:], scale=1.0 / SC)
                        nc.vector.scalar_tensor_tensor(
                            qfm_all[:, fo, c * CH:(c + 1) * CH],
                            ps[:], qb1[:, fo:fo + 1], e[:],
                            op0=add, op1=amax)

            # ---------------- Phase 2: attention + out-proj ---------------
            with (
                tc.tile_pool(name="dps", bufs=2, space="PSUM") as dps,
                tc.tile_pool(name="bcps", bufs=2, space="PSUM") as bcps,
                tc.tile_pool(name="attps", bufs=2, space="PSUM") as attps,
                tc.tile_pool(name="attp", bufs=2) as attp,
                tc.tile_pool(name="qsp", bufs=3) as qsp,
                tc.tile_pool(name="rcp", bufs=2) as rcp,
                tc.tile_pool(name="ypsum", bufs=2, space="PSUM") as yps,
                tc.tile_pool(name="ysb", bufs=2) as ysbp,
            ):
                for c in range(NCH):
                    csl = bass.ds(c * CH, CH)
                    # denominators for all 16 heads: [16, CH] (scaled S^2)
                    dn_ps = dps.tile([16, CH], F32, tag="dn")
                    for ec in range(EC):
                        nc.tensor.matmul(
                            dn_ps[:], ksb[:, ec, :], qfm_all[:, ec, csl],
                            start=(ec == 0), stop=(ec == EC - 1))
                    rec = rcp.tile([16, CH], F32R, tag="rec")
                    with nc.allow_low_precision(reason="f32r == f32 bits"):
                        nc.vector.reciprocal(rec[:], dn_ps[:])

                    att = attp.tile([128, EC, CH], BF16, tag="att")
                    for ec in range(EC):
                        # broadcast S_ATT/denom of the 2 heads to 128 parts
                        bc = bcps.tile([128, CH], F32, tag="bc")
                        nc.tensor.matmul(bc[:], ind[:, ec, :], rec[:],
                                         start=True, stop=True)
                        qs = qsp.tile([128, CH], BF16, tag="qs")
                        nc.vector.tensor_mul(qs[:], qfm_all[:, ec, csl], bc[:])
                        aps = attps.tile([128, CH], F32, tag="aps")
                        nc.tensor.matmul(aps[:], kvbd[:, ec, :], qs[:],
                                         start=True, stop=True)
                        nc.scalar.activation(att[:, ec, :], aps[:], Copy)

                    # out-projection: psum = S_O * y
                    for tb in range(TB):
                        ysb = ysbp.tile([128, E], F32, tag="ysb")
                        tsl = bass.ds(tb * 128, 128)
                        for fo in range(FC5):
                            fsl = bass.ds(fo * 512, 512)
                            py = yps.tile([128, 512], F32, tag="yps")
                            nc.tensor.matmul(py[:], k_ones[:],
                                             ob_row[:, fsl],
                                             start=True, stop=False)
                            for ec in range(EC):
                                nc.tensor.matmul(
                                    py[:],
                                    att[:, ec, tsl],
                                    owt[:, ec, fsl],
                                    start=False, stop=(ec == EC - 1))
                            nc.scalar.activation(ysb[:, fsl], py[:], Copy)
                        r0 = c * CH + tb * 128
                        nc.sync.dma_start(out=y[r0:r0 + 128, :], in_=ysb[:])

            _stack.close()

    nc.compile()
    return nc


_NC_CACHE = {}


def _get_nc(T, n_cores, groups):
    key = (T, n_cores, tuple(tuple(g) for g in groups))
    if key not in _NC_CACHE:
        _NC_CACHE[key] = build_nc(T, n_cores, groups)
    return _NC_CACHE[key]


def _tileize_in(w):
    # [A, B] -> [128, A//128, B] with out[p, a, b] = w[a*128+p, b]
    A, Bd = w.shape
    return np.ascontiguousarray(
        w.reshape(A // 128, 128, Bd).transpose(1, 0, 2))


def make_in_maps(inputs):
    """Host-side preprocessing: returns the per-core input maps."""
    import ml_dtypes
    bf16 = ml_dtypes.bfloat16
    fp8 = ml_dtypes.float8_e4m3

    query = np.asarray(inputs["query"], dtype=np.float32)
    key = np.asarray(inputs["key"], dtype=np.float32)
    value = np.asarray(inputs["value"], dtype=np.float32)
    b, s, e = query.shape
    assert (b, s, e) == (B, S, E)

    f32 = np.float32
    qd_w, qu_w, qu_b = (np.asarray(inputs[n], f32) for n in
                        ("qd_w", "qu_w", "qu_b"))
    kd_w, ku_w, ku_b = (np.asarray(inputs[n], f32) for n in
                        ("kd_w", "ku_w", "ku_b"))
    vd_w, vu_w, vu_b = (np.asarray(inputs[n], f32) for n in
                        ("vd_w", "vu_w", "vu_b"))
    out_w, out_b = (np.asarray(inputs[n], f32) for n in ("out_w", "out_b"))

    # combined projection weights: q = x @ (Wu Wd)^T + b
    # device wants wc[p, ec, f] = (Wu Wd)[f, ec*128+p] i.e. tileize of
    # (Wu Wd)^T = Wd^T Wu^T; scaled by SW for fp8
    wqc = _tileize_in(qd_w.T @ qu_w.T * SW).astype(fp8)
    wkc = _tileize_in(kd_w.T @ ku_w.T * SW).astype(fp8)
    wvc = _tileize_in(vd_w.T @ vu_w.T * SW).astype(fp8)
    owt = _tileize_in(out_w.T).astype(bf16)

    kb_row = (SC * (ku_b + 1.0)).reshape(1, E).astype(f32)
    ob_row = (S_O * out_b).reshape(1, E).astype(f32)
    qb0 = np.ascontiguousarray((SC * qu_b).reshape(E // 128, 128).T)
    qb1 = np.ascontiguousarray((SC * (qu_b + 1.0)).reshape(E // 128, 128).T)

    EC = E // 128
    # vbb[p, ec, e] = vu_b[head(p,ec)*D + e], head = 2*ec + (p>=64)
    vb_h = vu_b.reshape(H, D)
    vbb = np.empty((128, EC, D), dtype=f32)
    for ec in range(EC):
        vbb[0:64, ec, :] = vb_h[2 * ec]
        vbb[64:128, ec, :] = vb_h[2 * ec + 1]

    ind = np.zeros((16, EC, 128), dtype=f32)
    for ec in range(EC):
        ind[2 * ec, ec, 0:64] = S_ATT
        ind[2 * ec + 1, ec, 64:128] = S_ATT

    half = S // 2

    shared = {"wqc": wqc, "wkc": wkc, "wvc": wvc, "owT": owt,
              "kb_row": kb_row, "ob_row": ob_row, "qb0": qb0, "qb1": qb1,
              "vbb": vbb, "ind": ind,
              "lns": np.full((128, 1), LNS, dtype=f32),
              "ones_row": np.ones((1, 128), dtype=f32)}

    in_maps = []
    for c in range(N_CORES):
        bi, hi = c // 2, c % 2
        sl = slice(hi * half, (hi + 1) * half)
        m = {
            "xqT": np.ascontiguousarray(query[bi, sl].T * SX).astype(fp8),
            "xkT": np.ascontiguousarray(key[bi, sl].T * SX).astype(fp8),
            "xvT": np.ascontiguousarray(value[bi, sl].T * SX).astype(fp8),
        }
        m.update(shared)
        in_maps.append(m)

    return in_maps


def kernel(**inputs):
    in_maps = make_in_maps(inputs)
    groups = [[0, 1], [2, 3], [4, 5], [6, 7]]
    nc = _get_nc(B * S // N_CORES, N_CORES, groups)
    res = run_bass_kernel_spmd(nc, in_maps, list(range(N_CORES)))

    half = S // 2
    out = np.empty((B, S, E), dtype=np.float32)
    for c in range(N_CORES):
        bi, hi = c // 2, c % 2
        out[bi, hi * half:(hi + 1) * half] = res.results[c]["y"]
    return out
